# revision 10
# baseline (speedup 1.0000x reference)
# Bahdanau attention Trainium2 kernel (8-core SPMD, batch-parallel).
#
# reference:
#   q_proj = query @ w1 + b1                  [B, U]
#   v_proj = values @ w2 + b2                 [B, S, U]
#   score  = tanh(q_proj + v_proj) @ v + bv   [B, S, 1]
#   aw     = softmax(score masked)            [B, S, 1]
#   ctx    = sum(aw * values, axis=1)         [B, D]
#
# Shapes: B=32, S=2048, D=1024, U=1024.  8 cores, 4 batches/core.
#
# Per core, per batch:
#   phase 1: stream values[b] in s-chunks of 512, PE-transpose each
#            [128s x 128d] tile (fp32) into vT buffers with an fp32r
#            rounding copy (PSUM->SBUF), main matmul
#            v_projT[u,s] = w2.T @ vT in fp32r (full PE rate, TF32-ish
#            precision: 11 mantissa bits), fused bias+tanh on ACT
#            (written as fp32r), score matmul (v as [128,2] stationary —
#            fp32r requires even innermost stationary counts).
#   phase 2: masked softmax without max-subtraction (scores are tanh-
#            bounded by sum|v| ~ 26, exp(26) fine in fp32; masked lanes
#            multiplied by 0 matches the reference's exp(-1e9)=0).
#   phase 3: context matmul ctx = e.T @ values (s-contraction, natural
#            layout re-streamed from HBM + fp32r rounding copy), scaled
#            by 1/Z on the way out.
#
# bv is skipped: softmax is shift-invariant so it cannot affect outputs.

import sys

import numpy as np

if "/opt/trn_rl_repo" not in sys.path:
    sys.path.insert(0, "/opt/trn_rl_repo")

P = 128
B, S, D, U = 32, 2048, 1024, 1024
NCORES = 8
NB = B // NCORES          # batches per core
KD = D // P               # 8 d-tiles
JU = U // P               # 8 u-tiles
ST = S // P               # 16 s-tiles
SC = 512                  # s-chunk (free dim of main matmul / psum bank)
NCH = S // SC             # 4 chunks
TPC = SC // P             # 4 s-tiles per chunk
DH = 512                  # d-half for context matmul (psum bank limit)

_cache = {}


def build_module(mode="f32r"):
    """Build and compile the per-core Bass module. mode: 'f32r' | 'f32'."""
    if mode in _cache:
        return _cache[mode]

    from contextlib import ExitStack

    import concourse.bacc as bacc
    import concourse.tile as tile
    import concourse.mybir as mybir
    from concourse import masks

    dt = mybir.dt
    AF = mybir.ActivationFunctionType
    f32 = dt.float32
    rdt = dt.float32r if mode == "f32r" else dt.float32

    nc = bacc.Bacc("TRN2", target_bir_lowering=False, debug=False,
                   num_devices=NCORES)

    q_t = nc.dram_tensor("query", [NB, D], f32, kind="ExternalInput").ap()
    values_t = nc.dram_tensor("values", [NB, S, D], f32, kind="ExternalInput").ap()
    mask_t = nc.dram_tensor("mask", [NB, S], dt.int32, kind="ExternalInput").ap()
    w1_t = nc.dram_tensor("w1", [D, U], f32, kind="ExternalInput").ap()
    b1_t = nc.dram_tensor("b1", [U], f32, kind="ExternalInput").ap()
    w2_t = nc.dram_tensor("w2", [D, U], f32, kind="ExternalInput").ap()
    b2_t = nc.dram_tensor("b2", [U], f32, kind="ExternalInput").ap()
    v_t = nc.dram_tensor("v", [U, 1], f32, kind="ExternalInput").ap()
    nc.dram_tensor("bv", [1], f32, kind="ExternalInput")  # unused (shift-invariant)
    ctx_t = nc.dram_tensor("context_out", [NB, D], f32, kind="ExternalOutput").ap()
    aw_t = nc.dram_tensor("aw_out", [NB, S, 1], f32, kind="ExternalOutput").ap()
    scr_t = nc.dram_tensor("score_scratch", [NB, S], f32).ap()

    with tile.TileContext(nc) as tc, ExitStack() as es:
        const = es.enter_context(tc.tile_pool(name="const", bufs=1))
        psum = es.enter_context(tc.tile_pool(name="psum", bufs=1, space="PSUM"))
        natp = es.enter_context(tc.tile_pool(name="natp", bufs=1))
        vtp = es.enter_context(tc.tile_pool(name="vtp", bufs=1))
        tp = es.enter_context(tc.tile_pool(name="tp", bufs=1))
        smp = es.enter_context(tc.tile_pool(name="smp", bufs=1))

        # ---- constants / preamble ----
        ident = const.tile([P, P], f32, tag="ident")
        masks.make_identity(nc, ident)
        vsb = const.tile([P, JU], f32, tag="vsb")
        nc.sync.dma_start(out=vsb, in_=v_t.rearrange("(k p) o -> p (k o)", p=P))
        b1sb = const.tile([P, JU], f32, tag="b1sb")
        nc.sync.dma_start(out=b1sb, in_=b1_t.rearrange("(k p) -> p k", p=P))
        b2sb = const.tile([P, JU], f32, tag="b2sb")
        nc.sync.dma_start(out=b2sb, in_=b2_t.rearrange("(k p) -> p k", p=P))
        bias = const.tile([P, JU], f32, tag="bias")
        nc.vector.tensor_add(bias, b1sb, b2sb)
        qsb = const.tile([P, KD, NB], f32, tag="qsb")
        for bb in range(NB):
            nc.sync.dma_start(out=qsb[:, :, bb],
                              in_=q_t[bb].rearrange("(k p) -> p k", p=P))
        ones_p1 = const.tile([P, 1], f32, tag="ones_p1")
        nc.vector.memset(ones_p1, 1.0)
        ones_1p = const.tile([1, P], f32, tag="ones_1p")
        nc.vector.memset(ones_1p, 1.0)
        qb = const.tile([P, JU, NB], f32, tag="qb")

        # v as [128, 2] stationary columns (fp32r needs even innermost
        # counts on the stationary operand); odd columns zero.  fp32r
        # tiles cannot be memset directly (ISA), so zeros come from a
        # rounding copy of an fp32 zeros tile.
        zcol = const.tile([P, ST], f32, tag="zcol")
        nc.vector.memset(zcol, 0.0)
        vr2 = const.tile([P, 2 * JU], rdt, tag="vr2")
        vr2v = vr2.rearrange("p (j two) -> p j two", two=2)
        nc.vector.tensor_copy(vr2v[:, :, 0], vsb)
        nc.vector.tensor_copy(vr2v[:, :, 1], zcol[:, :JU])

        # w2 in fp32r, d-on-partition tiles: DMA raw chunk, rounding copy.
        w2p = es.enter_context(tc.tile_pool(name="w2p", bufs=1))
        w2sb = w2p.tile([P, KD, U], rdt, tag="w2")
        with ExitStack() as pre_scope:
            tmpp = pre_scope.enter_context(tc.tile_pool(name="tmpp", bufs=1))
            for k in range(KD):
                w2raw = tmpp.tile([P, U], f32, tag="w2raw", bufs=2)
                nc.sync.dma_start(out=w2raw, in_=w2_t[k * P:(k + 1) * P, :])
                nc.vector.tensor_copy(w2sb[:, k], w2raw)

            # q_proj for all NB batches at once (plain fp32 matmul), w1
            # streamed per k-tile.  All JU output groups live in one psum
            # bank: the j==0,k==0 matmul marks the bank pending-zero, every
            # other j's first write overwrites (per-element has_written),
            # k>0 accumulates.
            qp_all = psum.tile([P, JU * NB], f32, tag="sm", bufs=1)
            for k in range(KD):
                w1raw = tmpp.tile([P, U], f32, tag="w1raw", bufs=2)
                nc.sync.dma_start(out=w1raw, in_=w1_t[k * P:(k + 1) * P, :])
                for j in range(JU):
                    nc.tensor.matmul(qp_all[:, j * NB:(j + 1) * NB],
                                     w1raw[:, j * P:(j + 1) * P],
                                     qsb[:, k],
                                     start=(k == 0 and j == 0),
                                     stop=(k == KD - 1 and j == JU - 1),
                                     skip_group_check=True)
            for j in range(JU):
                nc.vector.tensor_scalar_add(qb[:, j],
                                            qp_all[:, j * NB:(j + 1) * NB],
                                            bias[:, j:j + 1])

        # ---- per-batch pipeline ----
        for b in range(NB):
            for c in range(NCH):
                nat = natp.tile([P, TPC, D], f32, tag="nat", bufs=3)
                nc.sync.dma_start(
                    out=nat,
                    in_=values_t[b, c * SC:(c + 1) * SC].rearrange(
                        "(i p) d -> p i d", p=P))

                vts = []
                for k in range(KD):
                    pt = psum.tile([P, SC], f32, tag="pt", bufs=2)
                    for i in range(TPC):
                        nc.tensor.transpose(pt[:, i * P:(i + 1) * P],
                                            nat[:, i, k * P:(k + 1) * P],
                                            ident)
                    vt = vtp.tile([P, SC], rdt, tag="vt", bufs=16)
                    if k % 2 == 0:
                        nc.vector.tensor_copy(vt, pt)
                    else:
                        nc.scalar.activation(vt, pt, AF.Copy)
                    vts.append(vt)

                sc_ps = psum.tile([2, SC], f32, tag="ps", bufs=1)
                for j in range(JU):
                    vp = psum.tile([P, SC], f32, tag="pv", bufs=2)
                    for k in range(KD):
                        nc.tensor.matmul(vp, w2sb[:, k, j * P:(j + 1) * P],
                                         vts[k],
                                         start=(k == 0), stop=(k == KD - 1))
                    tt = tp.tile([P, SC], rdt, tag="T", bufs=3)
                    nc.scalar.activation(tt, vp, AF.Tanh, bias=qb[:, j, b:b + 1])
                    nc.tensor.matmul(sc_ps, vr2[:, 2 * j:2 * j + 2], tt,
                                     start=(j == 0), stop=(j == JU - 1))
                score_flat = smp.tile([1, SC], f32, tag="scoref", bufs=2)
                nc.scalar.activation(score_flat, sc_ps[0:1, :], AF.Copy)
                nc.sync.dma_start(out=scr_t[b:b + 1, c * SC:(c + 1) * SC],
                                  in_=score_flat)

            # ---- softmax (no max-subtraction; scores bounded by sum|v|) ----
            score_sb = smp.tile([P, ST], f32, tag="ssb", bufs=2)
            nc.sync.dma_start(out=score_sb,
                              in_=scr_t[b].rearrange("(t p) -> p t", p=P))
            msb_i = smp.tile([P, ST], dt.int32, tag="mi", bufs=2)
            nc.sync.dma_start(out=msb_i,
                              in_=mask_t[b].rearrange("(t p) -> p t", p=P))
            msb = smp.tile([P, ST], f32, tag="mf", bufs=2)
            nc.vector.tensor_copy(msb, msb_i)
            e_sb = smp.tile([P, ST], f32, tag="esb", bufs=2)
            nc.scalar.activation(e_sb, score_sb, AF.Exp)
            nc.vector.tensor_mul(e_sb, e_sb, msb)
            # e as [128, 2] stationary columns for the context matmul.
            er2 = smp.tile([P, 2 * ST], rdt, tag="er2", bufs=2)
            er2v = er2.rearrange("p (t two) -> p t two", two=2)
            nc.vector.tensor_copy(er2v[:, :, 0], e_sb)
            nc.vector.tensor_copy(er2v[:, :, 1], zcol)
            rowsum = smp.tile([P, 1], f32, tag="rs", bufs=2)
            nc.vector.reduce_sum(rowsum, e_sb, axis=mybir.AxisListType.X)
            z_ps = psum.tile([1, 1], f32, tag="sm", bufs=1)
            nc.tensor.matmul(z_ps, rowsum, ones_p1, start=True, stop=True)
            zsb = smp.tile([1, 1], f32, tag="zsb", bufs=2)
            nc.vector.tensor_copy(zsb, z_ps)
            rz = smp.tile([1, 1], f32, tag="rz", bufs=2)
            nc.vector.reciprocal(rz, zsb)
            rzb_ps = psum.tile([P, 1], f32, tag="sm", bufs=1)
            nc.tensor.matmul(rzb_ps, ones_1p, rz, start=True, stop=True)
            rzb = smp.tile([P, 1], f32, tag="rzb", bufs=2)
            nc.vector.tensor_copy(rzb, rzb_ps)
            aw_sb = smp.tile([P, ST], f32, tag="awsb", bufs=2)
            nc.vector.tensor_scalar_mul(aw_sb, e_sb, rzb)
            nc.sync.dma_start(
                out=aw_t[b].rearrange("(t p) o -> p (t o)", p=P), in_=aw_sb)

            # ---- context = (e.T @ values) * (1/Z) ----
            ctx_ps = psum.tile([2, D], f32, tag="ctx", bufs=1)
            for c in range(NCH):
                nat2 = natp.tile([P, TPC, D], f32, tag="nat", bufs=3)
                nc.sync.dma_start(
                    out=nat2,
                    in_=values_t[b, c * SC:(c + 1) * SC].rearrange(
                        "(i p) d -> p i d", p=P))
                if rdt is not f32:
                    natr = natp.tile([P, TPC, D], rdt, tag="natr", bufs=2)
                    nc.vector.tensor_copy(natr, nat2)
                else:
                    natr = nat2
                for i in range(TPC):
                    t_idx = c * TPC + i
                    for h in range(2):
                        nc.tensor.matmul(
                            ctx_ps[:, h * DH:(h + 1) * DH],
                            er2[:, 2 * t_idx:2 * t_idx + 2],
                            natr[:, i, h * DH:(h + 1) * DH],
                            start=(t_idx == 0), stop=(t_idx == ST - 1))
            ctx_sb = smp.tile([1, D], f32, tag="ctxsb", bufs=1)
            nc.scalar.activation(ctx_sb, ctx_ps[0:1, :], AF.Copy, scale=rz)
            nc.sync.dma_start(out=ctx_t[b:b + 1], in_=ctx_sb)

    nc.compile()
    _cache[mode] = nc
    return nc


def _in_maps(query, values, mask, w1, b1, w2, b2, v, bv):
    query = np.ascontiguousarray(np.asarray(query, dtype=np.float32))
    values = np.ascontiguousarray(np.asarray(values, dtype=np.float32))
    mask = np.ascontiguousarray(np.asarray(mask, dtype=np.int32))
    w1 = np.ascontiguousarray(np.asarray(w1, dtype=np.float32))
    b1 = np.ascontiguousarray(np.asarray(b1, dtype=np.float32))
    w2 = np.ascontiguousarray(np.asarray(w2, dtype=np.float32))
    b2 = np.ascontiguousarray(np.asarray(b2, dtype=np.float32))
    v = np.ascontiguousarray(np.asarray(v, dtype=np.float32))
    bv = np.ascontiguousarray(np.asarray(bv, dtype=np.float32))
    maps = []
    for c in range(NCORES):
        sl = slice(c * NB, (c + 1) * NB)
        maps.append({
            "query": query[sl], "values": values[sl], "mask": mask[sl],
            "w1": w1, "b1": b1, "w2": w2, "b2": b2, "v": v, "bv": bv,
        })
    return maps


def kernel(query, values, mask, w1, b1, w2, b2, v, bv, _trace=False, _mode="f32r"):
    from concourse import bass_utils

    nc = build_module(_mode)
    maps = _in_maps(query, values, mask, w1, b1, w2, b2, v, bv)
    res = bass_utils.run_bass_kernel_spmd(
        nc, maps, core_ids=list(range(NCORES)), trace=_trace)
    ctx = np.concatenate([r["context_out"] for r in res.results], axis=0)
    aw = np.concatenate([r["aw_out"] for r in res.results], axis=0)
    if _trace:
        kernel.last_results = res
    return ctx, aw


# revision 11
# speedup vs baseline: 7.4765x; 7.4765x over previous
# Bahdanau attention Trainium2 kernel (8-core SPMD, batch-parallel).
#
# reference:
#   q_proj = query @ w1 + b1                  [B, U]
#   v_proj = values @ w2 + b2                 [B, S, U]
#   score  = tanh(q_proj + v_proj) @ v + bv   [B, S, 1]
#   aw     = softmax(score masked)            [B, S, 1]
#   ctx    = sum(aw * values, axis=1)         [B, D]
#
# Shapes: B=32, S=2048, D=1024, U=1024.  8 cores, 4 batches/core.
#
# Per core, per batch:
#   phase 1: stream values[b] in s-chunks of 512, PE-transpose each
#            [128s x 128d] tile (fp32) into vT buffers with an fp32r
#            rounding copy (PSUM->SBUF), main matmul
#            v_projT[u,s] = w2.T @ vT in fp32r (full PE rate, TF32-ish
#            precision: 11 mantissa bits), fused bias+tanh on ACT
#            (written as fp32r), score matmul (v as [128,2] stationary —
#            fp32r requires even innermost stationary counts).
#   phase 2: masked softmax without max-subtraction (scores are tanh-
#            bounded by sum|v| ~ 26, exp(26) fine in fp32; masked lanes
#            multiplied by 0 matches the reference's exp(-1e9)=0).
#   phase 3: context matmul ctx = e.T @ values (s-contraction, natural
#            layout re-streamed from HBM + fp32r rounding copy), scaled
#            by 1/Z on the way out.
#
# bv is skipped: softmax is shift-invariant so it cannot affect outputs.

import sys

import numpy as np

if "/opt/trn_rl_repo" not in sys.path:
    sys.path.insert(0, "/opt/trn_rl_repo")

P = 128
B, S, D, U = 32, 2048, 1024, 1024
NCORES = 8
NB = B // NCORES          # batches per core
KD = D // P               # 8 d-tiles
JU = U // P               # 8 u-tiles
ST = S // P               # 16 s-tiles
SC = 512                  # s-chunk (free dim of main matmul / psum bank)
NCH = S // SC             # 4 chunks
TPC = SC // P             # 4 s-tiles per chunk
DH = 512                  # d-half for context matmul (psum bank limit)

_cache = {}


def build_module(mode="f32r", repeat=1):
    """Build and compile the per-core Bass module. mode: 'f32r' | 'f32'.

    repeat > 1 re-emits the whole per-batch pipeline R times (same
    outputs overwritten) — used by bench.py to isolate device time from
    dispatch overhead via the marginal cost of a repeat."""
    key = (mode, repeat)
    if key in _cache:
        return _cache[key]

    from contextlib import ExitStack

    import concourse.bacc as bacc
    import concourse.tile as tile
    import concourse.mybir as mybir
    from concourse import masks

    dt = mybir.dt
    AF = mybir.ActivationFunctionType
    f32 = dt.float32
    rdt = dt.float32r if mode == "f32r" else dt.float32

    nc = bacc.Bacc("TRN2", target_bir_lowering=False, debug=False,
                   num_devices=NCORES)

    q_t = nc.dram_tensor("query", [NB, D], f32, kind="ExternalInput").ap()
    values_t = nc.dram_tensor("values", [NB, S, D], f32, kind="ExternalInput").ap()
    mask_t = nc.dram_tensor("mask", [NB, S], dt.int32, kind="ExternalInput").ap()
    w1_t = nc.dram_tensor("w1", [D, U], f32, kind="ExternalInput").ap()
    b1_t = nc.dram_tensor("b1", [U], f32, kind="ExternalInput").ap()
    w2_t = nc.dram_tensor("w2", [D, U], f32, kind="ExternalInput").ap()
    b2_t = nc.dram_tensor("b2", [U], f32, kind="ExternalInput").ap()
    v_t = nc.dram_tensor("v", [U, 1], f32, kind="ExternalInput").ap()
    nc.dram_tensor("bv", [1], f32, kind="ExternalInput")  # unused (shift-invariant)
    ctx_t = nc.dram_tensor("context_out", [NB, D], f32, kind="ExternalOutput").ap()
    aw_t = nc.dram_tensor("aw_out", [NB, S, 1], f32, kind="ExternalOutput").ap()
    scr_t = nc.dram_tensor("score_scratch", [NB, S], f32).ap()

    with tile.TileContext(nc) as tc, ExitStack() as es:
        const = es.enter_context(tc.tile_pool(name="const", bufs=1))
        psum = es.enter_context(tc.tile_pool(name="psum", bufs=1, space="PSUM"))
        natp = es.enter_context(tc.tile_pool(name="natp", bufs=1))
        vtp = es.enter_context(tc.tile_pool(name="vtp", bufs=1))
        tp = es.enter_context(tc.tile_pool(name="tp", bufs=1))
        smp = es.enter_context(tc.tile_pool(name="smp", bufs=1))

        # ---- constants / preamble ----
        ident = const.tile([P, P], f32, tag="ident")
        masks.make_identity(nc, ident)
        vsb = const.tile([P, JU], f32, tag="vsb")
        nc.sync.dma_start(out=vsb, in_=v_t.rearrange("(k p) o -> p (k o)", p=P))
        b1sb = const.tile([P, JU], f32, tag="b1sb")
        nc.sync.dma_start(out=b1sb, in_=b1_t.rearrange("(k p) -> p k", p=P))
        b2sb = const.tile([P, JU], f32, tag="b2sb")
        nc.sync.dma_start(out=b2sb, in_=b2_t.rearrange("(k p) -> p k", p=P))
        bias = const.tile([P, JU], f32, tag="bias")
        nc.vector.tensor_add(bias, b1sb, b2sb)
        qsb = const.tile([P, KD, NB], f32, tag="qsb")
        for bb in range(NB):
            nc.sync.dma_start(out=qsb[:, :, bb],
                              in_=q_t[bb].rearrange("(k p) -> p k", p=P))
        ones_p1 = const.tile([P, 1], f32, tag="ones_p1")
        nc.vector.memset(ones_p1, 1.0)
        ones_1p = const.tile([1, P], f32, tag="ones_1p")
        nc.vector.memset(ones_1p, 1.0)
        qb = const.tile([P, JU, NB], f32, tag="qb")

        # v as [128, 2] stationary columns (fp32r needs even innermost
        # counts on the stationary operand); odd columns zero.  fp32r
        # tiles cannot be memset directly (ISA), so zeros come from a
        # rounding copy of an fp32 zeros tile.
        zcol = const.tile([P, ST], f32, tag="zcol")
        nc.vector.memset(zcol, 0.0)
        vr2 = const.tile([P, 2 * JU], rdt, tag="vr2")
        vr2v = vr2.rearrange("p (j two) -> p j two", two=2)
        nc.vector.tensor_copy(vr2v[:, :, 0], vsb)
        nc.vector.tensor_copy(vr2v[:, :, 1], zcol[:, :JU])

        # w2 in fp32r, d-on-partition tiles: DMA raw chunk, rounding copy.
        w2p = es.enter_context(tc.tile_pool(name="w2p", bufs=1))
        w2sb = w2p.tile([P, KD, U], rdt, tag="w2")
        with ExitStack() as pre_scope:
            tmpp = pre_scope.enter_context(tc.tile_pool(name="tmpp", bufs=1))
            for k in range(KD):
                w2raw = tmpp.tile([P, U], f32, tag="w2raw", bufs=2)
                nc.sync.dma_start(out=w2raw, in_=w2_t[k * P:(k + 1) * P, :])
                nc.vector.tensor_copy(w2sb[:, k], w2raw)

            # q_proj for all NB batches at once (plain fp32 matmul), w1
            # streamed per k-tile.  All JU output groups live in one psum
            # bank: the j==0,k==0 matmul marks the bank pending-zero, every
            # other j's first write overwrites (per-element has_written),
            # k>0 accumulates.
            qp_all = psum.tile([P, JU * NB], f32, tag="sm", bufs=1)
            for k in range(KD):
                w1raw = tmpp.tile([P, U], f32, tag="w1raw", bufs=2)
                nc.sync.dma_start(out=w1raw, in_=w1_t[k * P:(k + 1) * P, :])
                for j in range(JU):
                    nc.tensor.matmul(qp_all[:, j * NB:(j + 1) * NB],
                                     w1raw[:, j * P:(j + 1) * P],
                                     qsb[:, k],
                                     start=(k == 0 and j == 0),
                                     stop=(k == KD - 1 and j == JU - 1),
                                     skip_group_check=True)
            for j in range(JU):
                nc.vector.tensor_scalar_add(qb[:, j],
                                            qp_all[:, j * NB:(j + 1) * NB],
                                            bias[:, j:j + 1])

        # ---- per-batch pipeline ----
        for b in [bi for _ in range(repeat) for bi in range(NB)]:
            for c in range(NCH):
                nat = natp.tile([P, TPC, D], f32, tag="nat", bufs=3)
                nc.sync.dma_start(
                    out=nat,
                    in_=values_t[b, c * SC:(c + 1) * SC].rearrange(
                        "(i p) d -> p i d", p=P))

                vts = []
                for k in range(KD):
                    pt = psum.tile([P, SC], f32, tag="pt", bufs=2)
                    for i in range(TPC):
                        nc.tensor.transpose(pt[:, i * P:(i + 1) * P],
                                            nat[:, i, k * P:(k + 1) * P],
                                            ident)
                    vt = vtp.tile([P, SC], rdt, tag="vt", bufs=16)
                    if k % 2 == 0:
                        nc.vector.tensor_copy(vt, pt)
                    else:
                        nc.scalar.activation(vt, pt, AF.Copy)
                    vts.append(vt)

                sc_ps = psum.tile([2, SC], f32, tag="ps", bufs=1)
                for j in range(JU):
                    vp = psum.tile([P, SC], f32, tag="pv", bufs=2)
                    for k in range(KD):
                        nc.tensor.matmul(vp, w2sb[:, k, j * P:(j + 1) * P],
                                         vts[k],
                                         start=(k == 0), stop=(k == KD - 1))
                    tt = tp.tile([P, SC], rdt, tag="T", bufs=3)
                    nc.scalar.activation(tt, vp, AF.Tanh, bias=qb[:, j, b:b + 1])
                    nc.tensor.matmul(sc_ps, vr2[:, 2 * j:2 * j + 2], tt,
                                     start=(j == 0), stop=(j == JU - 1))
                score_flat = smp.tile([1, SC], f32, tag="scoref", bufs=2)
                nc.scalar.activation(score_flat, sc_ps[0:1, :], AF.Copy)
                nc.sync.dma_start(out=scr_t[b:b + 1, c * SC:(c + 1) * SC],
                                  in_=score_flat)

            # ---- softmax (no max-subtraction; scores bounded by sum|v|) ----
            score_sb = smp.tile([P, ST], f32, tag="ssb", bufs=2)
            nc.sync.dma_start(out=score_sb,
                              in_=scr_t[b].rearrange("(t p) -> p t", p=P))
            msb_i = smp.tile([P, ST], dt.int32, tag="mi", bufs=2)
            nc.sync.dma_start(out=msb_i,
                              in_=mask_t[b].rearrange("(t p) -> p t", p=P))
            msb = smp.tile([P, ST], f32, tag="mf", bufs=2)
            nc.vector.tensor_copy(msb, msb_i)
            e_sb = smp.tile([P, ST], f32, tag="esb", bufs=2)
            nc.scalar.activation(e_sb, score_sb, AF.Exp)
            nc.vector.tensor_mul(e_sb, e_sb, msb)
            # e as [128, 2] stationary columns for the context matmul.
            er2 = smp.tile([P, 2 * ST], rdt, tag="er2", bufs=2)
            er2v = er2.rearrange("p (t two) -> p t two", two=2)
            nc.vector.tensor_copy(er2v[:, :, 0], e_sb)
            nc.vector.tensor_copy(er2v[:, :, 1], zcol)
            rowsum = smp.tile([P, 1], f32, tag="rs", bufs=2)
            nc.vector.reduce_sum(rowsum, e_sb, axis=mybir.AxisListType.X)
            z_ps = psum.tile([1, 1], f32, tag="sm", bufs=1)
            nc.tensor.matmul(z_ps, rowsum, ones_p1, start=True, stop=True)
            zsb = smp.tile([1, 1], f32, tag="zsb", bufs=2)
            nc.vector.tensor_copy(zsb, z_ps)
            rz = smp.tile([1, 1], f32, tag="rz", bufs=2)
            nc.vector.reciprocal(rz, zsb)
            rzb_ps = psum.tile([P, 1], f32, tag="sm", bufs=1)
            nc.tensor.matmul(rzb_ps, ones_1p, rz, start=True, stop=True)
            rzb = smp.tile([P, 1], f32, tag="rzb", bufs=2)
            nc.vector.tensor_copy(rzb, rzb_ps)
            aw_sb = smp.tile([P, ST], f32, tag="awsb", bufs=2)
            nc.vector.tensor_scalar_mul(aw_sb, e_sb, rzb)
            nc.sync.dma_start(
                out=aw_t[b].rearrange("(t p) o -> p (t o)", p=P), in_=aw_sb)

            # ---- context = (e.T @ values) * (1/Z) ----
            ctx_ps = psum.tile([2, D], f32, tag="ctx", bufs=1)
            for c in range(NCH):
                nat2 = natp.tile([P, TPC, D], f32, tag="nat", bufs=3)
                nc.sync.dma_start(
                    out=nat2,
                    in_=values_t[b, c * SC:(c + 1) * SC].rearrange(
                        "(i p) d -> p i d", p=P))
                if rdt is not f32:
                    natr = natp.tile([P, TPC, D], rdt, tag="natr", bufs=2)
                    nc.vector.tensor_copy(natr, nat2)
                else:
                    natr = nat2
                for i in range(TPC):
                    t_idx = c * TPC + i
                    for h in range(2):
                        nc.tensor.matmul(
                            ctx_ps[:, h * DH:(h + 1) * DH],
                            er2[:, 2 * t_idx:2 * t_idx + 2],
                            natr[:, i, h * DH:(h + 1) * DH],
                            start=(t_idx == 0), stop=(t_idx == ST - 1))
            ctx_sb = smp.tile([1, D], f32, tag="ctxsb", bufs=1)
            nc.scalar.activation(ctx_sb, ctx_ps[0:1, :], AF.Copy, scale=rz)
            nc.sync.dma_start(out=ctx_t[b:b + 1], in_=ctx_sb)

    nc.compile()
    _cache[key] = nc
    return nc


def _in_maps(query, values, mask, w1, b1, w2, b2, v, bv):
    query = np.ascontiguousarray(np.asarray(query, dtype=np.float32))
    values = np.ascontiguousarray(np.asarray(values, dtype=np.float32))
    mask = np.ascontiguousarray(np.asarray(mask, dtype=np.int32))
    w1 = np.ascontiguousarray(np.asarray(w1, dtype=np.float32))
    b1 = np.ascontiguousarray(np.asarray(b1, dtype=np.float32))
    w2 = np.ascontiguousarray(np.asarray(w2, dtype=np.float32))
    b2 = np.ascontiguousarray(np.asarray(b2, dtype=np.float32))
    v = np.ascontiguousarray(np.asarray(v, dtype=np.float32))
    bv = np.ascontiguousarray(np.asarray(bv, dtype=np.float32))
    maps = []
    for c in range(NCORES):
        sl = slice(c * NB, (c + 1) * NB)
        maps.append({
            "query": query[sl], "values": values[sl], "mask": mask[sl],
            "w1": w1, "b1": b1, "w2": w2, "b2": b2, "v": v, "bv": bv,
        })
    return maps


def kernel(query, values, mask, w1, b1, w2, b2, v, bv, _trace=False, _mode="f32r"):
    from concourse import bass_utils

    nc = build_module(_mode)
    maps = _in_maps(query, values, mask, w1, b1, w2, b2, v, bv)
    res = bass_utils.run_bass_kernel_spmd(
        nc, maps, core_ids=list(range(NCORES)), trace=_trace)
    ctx = np.concatenate([r["context_out"] for r in res.results], axis=0)
    aw = np.concatenate([r["aw_out"] for r in res.results], axis=0)
    if _trace:
        kernel.last_results = res
    return ctx, aw


# revision 26
# speedup vs baseline: 15.9920x; 2.1390x over previous
# Bahdanau attention Trainium2 kernel (8-core SPMD, batch-parallel).
#
# reference:
#   q_proj = query @ w1 + b1                  [B, U]
#   v_proj = values @ w2 + b2                 [B, S, U]
#   score  = tanh(q_proj + v_proj) @ v + bv   [B, S, 1]
#   aw     = softmax(score masked)            [B, S, 1]
#   ctx    = sum(aw * values, axis=1)         [B, D]
#
# Shapes: B=32, S=2048, D=1024, U=1024.  8 cores, 4 batches/core.
#
# Per core, per batch:
#   phase 1: stream values[b] in s-chunks of 512, PE-transpose each
#            [128s x 128d] tile (fp32) into vT buffers with an fp32r
#            rounding copy (PSUM->SBUF), main matmul
#            v_projT[u,s] = w2.T @ vT in fp32r (full PE rate, TF32-ish
#            precision: 11 mantissa bits), fused bias+tanh on ACT
#            (written as fp32r), score matmul (v as [128,2] stationary —
#            fp32r requires even innermost stationary counts).
#   phase 2: masked softmax without max-subtraction (scores are tanh-
#            bounded by sum|v| ~ 26, exp(26) fine in fp32; masked lanes
#            multiplied by 0 matches the reference's exp(-1e9)=0).
#   phase 3: context matmul ctx = e.T @ values (s-contraction, natural
#            layout re-streamed from HBM + fp32r rounding copy), scaled
#            by 1/Z on the way out.
#
# bv is skipped: softmax is shift-invariant so it cannot affect outputs.

import sys

import numpy as np

if "/opt/trn_rl_repo" not in sys.path:
    sys.path.insert(0, "/opt/trn_rl_repo")

P = 128
B, S, D, U = 32, 2048, 1024, 1024
NCORES = 8
NB = B // NCORES          # batches per core
KD = D // P               # 8 d-tiles
JU = U // P               # 8 u-tiles
ST = S // P               # 16 s-tiles
SC = 512                  # s-chunk (free dim of main matmul / psum bank)
NCH = S // SC             # 4 chunks
TPC = SC // P             # 4 s-tiles per chunk
DH = 512                  # d-half for context matmul (psum bank limit)

_cache = {}


def build_module(mode="f32r", repeat=1, cfg=None):
    """Build and compile the per-core Bass module. mode: 'f32r' | 'f32'.

    repeat > 1 re-emits the whole per-batch pipeline R times (same
    outputs overwritten) — used by bench.py to isolate device time from
    dispatch overhead via the marginal cost of a repeat."""
    cfg = dict(cfg or {})
    nat_bufs = cfg.pop("nat_bufs", 2)
    natr_bufs = cfg.pop("natr_bufs", 4)
    vt_bufs = cfg.pop("vt_bufs", 16)
    t_bufs = cfg.pop("t_bufs", 3)
    pt_bufs = cfg.pop("pt_bufs", 2)
    pv_bufs = cfg.pop("pv_bufs", 2)
    vt_eng = cfg.pop("vt_eng", "dve")   # 'dve' | 'act' | 'mix'
    qproj_late = cfg.pop("qproj_late", False)
    split_nat = cfg.pop("split_nat", True)
    preorder = cfg.pop("preorder", "cur")  # 'cur' | 'w1first' | 'w1c0'
    ctx_lag = cfg.pop("ctx_lag", 3)
    assert not cfg, cfg
    key = (mode, repeat, nat_bufs, natr_bufs, vt_bufs, t_bufs, pt_bufs,
           pv_bufs, vt_eng, qproj_late, split_nat, preorder, ctx_lag)
    if key in _cache:
        return _cache[key]

    from contextlib import ExitStack

    import concourse.bacc as bacc
    import concourse.tile as tile
    import concourse.mybir as mybir
    from concourse import masks

    dt = mybir.dt
    AF = mybir.ActivationFunctionType
    f32 = dt.float32
    rdt = dt.float32r if mode == "f32r" else dt.float32

    nc = bacc.Bacc("TRN2", target_bir_lowering=False, debug=False,
                   num_devices=NCORES)

    q_t = nc.dram_tensor("query", [NB, D], f32, kind="ExternalInput").ap()
    values_t = nc.dram_tensor("values", [NB, S, D], f32, kind="ExternalInput").ap()
    mask_t = nc.dram_tensor("mask", [NB, S], dt.int32, kind="ExternalInput").ap()
    w1_t = nc.dram_tensor("w1", [D, U], f32, kind="ExternalInput").ap()
    b1_t = nc.dram_tensor("b1", [U], f32, kind="ExternalInput").ap()
    w2_t = nc.dram_tensor("w2", [D, U], f32, kind="ExternalInput").ap()
    b2_t = nc.dram_tensor("b2", [U], f32, kind="ExternalInput").ap()
    v_t = nc.dram_tensor("v", [U, 1], f32, kind="ExternalInput").ap()
    nc.dram_tensor("bv", [1], f32, kind="ExternalInput")  # unused (shift-invariant)
    ctx_t = nc.dram_tensor("context_out", [NB, D], f32, kind="ExternalOutput").ap()
    aw_t = nc.dram_tensor("aw_out", [NB, S, 1], f32, kind="ExternalOutput").ap()
    scr_t = nc.dram_tensor("score_scratch", [NB, S], f32).ap()

    with tile.TileContext(nc) as tc, ExitStack() as es:
        const = es.enter_context(tc.tile_pool(name="const", bufs=1))
        psum = es.enter_context(tc.tile_pool(name="psum", bufs=1, space="PSUM"))
        natp = es.enter_context(tc.tile_pool(name="natp", bufs=1))
        vtp = es.enter_context(tc.tile_pool(name="vtp", bufs=1))
        tp = es.enter_context(tc.tile_pool(name="tp", bufs=1))
        smp = es.enter_context(tc.tile_pool(name="smp", bufs=1))

        # ---- constants / preamble ----
        ident = const.tile([P, P], f32, tag="ident")
        masks.make_identity(nc, ident)
        if rdt is not f32:
            identr = const.tile([P, P], rdt, tag="identr")
            nc.vector.tensor_copy(identr, ident)
        else:
            identr = ident
        vsb = const.tile([P, JU], f32, tag="vsb")
        nc.sync.dma_start(out=vsb, in_=v_t.rearrange("(k p) o -> p (k o)", p=P))
        b1sb = const.tile([P, JU], f32, tag="b1sb")
        nc.sync.dma_start(out=b1sb, in_=b1_t.rearrange("(k p) -> p k", p=P))
        b2sb = const.tile([P, JU], f32, tag="b2sb")
        nc.sync.dma_start(out=b2sb, in_=b2_t.rearrange("(k p) -> p k", p=P))
        bias = const.tile([P, JU], f32, tag="bias")
        nc.vector.tensor_add(bias, b1sb, b2sb)
        qsb = const.tile([P, KD, NB], f32, tag="qsb")
        for bb in range(NB):
            nc.sync.dma_start(out=qsb[:, :, bb],
                              in_=q_t[bb].rearrange("(k p) -> p k", p=P))
        ones_p1 = const.tile([P, 1], f32, tag="ones_p1")
        nc.vector.memset(ones_p1, 1.0)
        ones_1p = const.tile([1, P], f32, tag="ones_1p")
        nc.vector.memset(ones_1p, 1.0)
        qb = const.tile([P, JU, NB], f32, tag="qb")

        # v as [128, 2] stationary columns (fp32r needs even innermost
        # counts on the stationary operand); odd columns zero.  fp32r
        # tiles cannot be memset directly (ISA), so zeros come from a
        # rounding copy of an fp32 zeros tile.
        zcol = const.tile([P, ST], f32, tag="zcol")
        nc.vector.memset(zcol, 0.0)
        vr2 = const.tile([P, 2 * JU], rdt, tag="vr2")
        vr2v = vr2.rearrange("p (j two) -> p j two", two=2)
        nc.vector.tensor_copy(vr2v[:, :, 0], vsb)
        nc.vector.tensor_copy(vr2v[:, :, 1], zcol[:, :JU])

        # prefetch the first two values chunks before the 8 MB of weight
        # DMAs so the PE's transpose pipeline starts immediately.
        def emit_nat_dma(b, c):
            # raw fp32 staging DMA + fp32r rounding copy; the rounded tile
            # feeds both the PE transposes (phase 1) and, kept resident,
            # the context matmul (phase 3) — values are read from HBM once
            # per batch and rounded once.
            nat = natp.tile([P, TPC, D], f32, tag="nat", bufs=nat_bufs)
            src_ap = values_t[b, c * SC:(c + 1) * SC].rearrange(
                "(i p) d -> p i d", p=P)
            if split_nat:
                nc.sync.dma_start(out=nat[:, :2], in_=src_ap[:, :2])
                nc.sync.dma_start(out=nat[:, 2:], in_=src_ap[:, 2:])
            else:
                nc.sync.dma_start(out=nat, in_=src_ap)
            if rdt is not f32:
                natr = natp.tile([P, TPC, D], rdt, tag="natr", bufs=natr_bufs)
                nc.vector.tensor_copy(natr[:, :2], nat[:, :2])
                nc.vector.tensor_copy(natr[:, 2:], nat[:, 2:])
                return natr
            return nat


        # w2 in fp32r, d-on-partition tiles: DMA raw chunk, rounding copy.
        w2p = es.enter_context(tc.tile_pool(name="w2p", bufs=1))
        w2sb = w2p.tile([P, KD, U], rdt, tag="w2")
        tmpp_scope = ExitStack()
        tmpp = tmpp_scope.enter_context(tc.tile_pool(name="tmpp", bufs=1))

        def emit_w2_load():
            emit_w2_half(0)
            emit_w2_half(1)

        def emit_qproj():
            # q_proj for all NB batches at once (plain fp32 matmul), w1
            # streamed per k-tile.  All JU output groups live in one psum
            # bank: the j==0,k==0 matmul marks the bank pending-zero, every
            # other j's first write overwrites (per-element has_written),
            # k>0 accumulates.
            qp_all = psum.tile([P, JU * NB], f32, tag="sm", bufs=1)
            for k in range(KD):
                w1raw = tmpp.tile([P, U], f32, tag="w1raw", bufs=2)
                nc.sync.dma_start(out=w1raw, in_=w1_t[k * P:(k + 1) * P, :])
                for j in range(JU):
                    nc.tensor.matmul(qp_all[:, j * NB:(j + 1) * NB],
                                     w1raw[:, j * P:(j + 1) * P],
                                     qsb[:, k],
                                     start=(k == 0 and j == 0),
                                     stop=(k == KD - 1 and j == JU - 1),
                                     skip_group_check=True)
            for j in range(JU):
                nc.vector.tensor_scalar_add(qb[:, j],
                                            qp_all[:, j * NB:(j + 1) * NB],
                                            bias[:, j:j + 1])

        def emit_w2_half(h):
            for k in range(KD):
                w2raw = tmpp.tile([P, DH], f32, tag="w2raw", bufs=2)
                nc.sync.dma_start(
                    out=w2raw,
                    in_=w2_t[k * P:(k + 1) * P, h * DH:(h + 1) * DH])
                nc.vector.tensor_copy(w2sb[:, k, h * DH:(h + 1) * DH], w2raw)

        if preorder == "cur":
            prefetch_nat0 = emit_nat_dma(0, 0)
            prefetch_nat1 = emit_nat_dma(0, 1)
            emit_w2_load()
            if not qproj_late:
                emit_qproj()
                tmpp_scope.close()
        elif preorder == "w1first":
            emit_qproj()
            prefetch_nat0 = emit_nat_dma(0, 0)
            prefetch_nat1 = emit_nat_dma(0, 1)
            emit_w2_load()
            tmpp_scope.close()
        elif preorder == "w1c0":
            emit_qproj()
            prefetch_nat0 = emit_nat_dma(0, 0)
            emit_w2_half(0)
            prefetch_nat1 = emit_nat_dma(0, 1)
            emit_w2_half(1)
            tmpp_scope.close()
        else:
            raise ValueError(preorder)

        # ---- per-batch pipeline, software-pipelined emission ----
        # Tile freezes per-engine program order at emission time, so the
        # emission order IS the schedule skeleton:
        #  * transposes of chunk c+1 are interleaved with the matmul
        #    chains of chunk c (PE order: T(c+1,k) MM(c,j) T(c+1,k+1)
        #    MM(c,j+1)...), which hides the PSUM->SBUF vT copies;
        #  * batch b's softmax (DRAM score round-trip) and context
        #    matmuls are emitted inside batch b+1's phase 1 so they
        #    never head-of-line-block independent work.

        def emit_transpose_k(nat, k):
            pt = psum.tile([P, SC], f32, tag="pt", bufs=pt_bufs)
            for i in range(TPC):
                nc.tensor.transpose(
                    pt[:, i * P:(i + 1) * P].bitcast(rdt),
                    nat[:, i, k * P:(k + 1) * P],
                    identr)
            vt = vtp.tile([P, SC], rdt, tag="vt", bufs=vt_bufs)
            use_dve = (vt_eng == "dve" or (vt_eng == "mix" and k % 2 == 0))
            if use_dve:
                nc.vector.tensor_copy(vt, pt)
            else:
                nc.scalar.activation(vt, pt, AF.Copy)
            return vt

        def emit_mm_j(b, j, vts, sc_ps):
            vp = psum.tile([P, SC], f32, tag="pv", bufs=pv_bufs)
            for k in range(KD):
                nc.tensor.matmul(vp, w2sb[:, k, j * P:(j + 1) * P], vts[k],
                                 start=(k == 0), stop=(k == KD - 1))
            tt = tp.tile([P, SC], rdt, tag="T", bufs=t_bufs)
            nc.scalar.activation(tt, vp, AF.Tanh, bias=qb[:, j, b:b + 1])
            nc.tensor.matmul(sc_ps, vr2[:, 2 * j:2 * j + 2], tt,
                             start=(j == 0), stop=(j == JU - 1))

        def emit_score_finish(b, c, sc_ps, bst):
            # score chunk -> DRAM scratch -> reload s-on-partitions,
            # exp+mask immediately (softmax is computed incrementally per
            # chunk; no max-subtraction: scores are tanh-bounded ~26).
            score_flat = smp.tile([1, SC], f32, tag="scoref", bufs=2)
            nc.scalar.activation(score_flat, sc_ps[0:1, :], AF.Copy)
            nc.sync.dma_start(out=scr_t[b:b + 1, c * SC:(c + 1) * SC],
                              in_=score_flat)
            if c == 0:
                bst["e_sb"] = smp.tile([P, ST], f32, tag="esb", bufs=2, name="e_sb")
                bst["er2"] = smp.tile([P, 2 * ST], rdt, tag="er2", bufs=2, name="er2")
            score_sb = smp.tile([P, TPC], f32, tag="ssb", bufs=2)
            nc.sync.dma_start(
                out=score_sb,
                in_=scr_t[b, c * SC:(c + 1) * SC].rearrange(
                    "(t p) -> p t", p=P))
            msb_i = smp.tile([P, TPC], dt.int32, tag="mi", bufs=2)
            nc.sync.dma_start(
                out=msb_i,
                in_=mask_t[b, c * SC:(c + 1) * SC].rearrange(
                    "(t p) -> p t", p=P))
            msb = smp.tile([P, TPC], f32, tag="mf", bufs=2)
            nc.vector.tensor_copy(msb, msb_i)
            e_c = bst["e_sb"][:, c * TPC:(c + 1) * TPC]
            nc.scalar.activation(e_c, score_sb, AF.Exp)
            nc.vector.tensor_mul(e_c, e_c, msb)
            er2v = bst["er2"].rearrange("p (t two) -> p t two", two=2)
            nc.vector.tensor_copy(er2v[:, c * TPC:(c + 1) * TPC, 0], e_c)
            nc.vector.tensor_copy(er2v[:, c * TPC:(c + 1) * TPC, 1],
                                  zcol[:, :TPC])

        def emit_ctx_chunk(b, c, bst):
            # context accumulation for one chunk (unnormalized e weights;
            # the 1/Z scale happens once at batch end).
            if c == 0:
                bst["ctx_ps"] = psum.tile([2, D], f32, tag="ctx", bufs=1, name="ctx_ps")
            er2, ctx_ps = bst["er2"], bst["ctx_ps"]
            natr = emit_nat_dma(b, c)
            for i in range(TPC):
                t_idx = c * TPC + i
                for h in range(2):
                    nc.tensor.matmul(
                        ctx_ps[:, h * DH:(h + 1) * DH],
                        er2[:, 2 * t_idx:2 * t_idx + 2],
                        natr[:, i, h * DH:(h + 1) * DH],
                        start=(t_idx == 0), stop=(t_idx == ST - 1))

        def emit_batch_finish(b, bst):
            # Z, 1/Z, attention-weight output, scaled context output.
            e_sb = bst["e_sb"]
            rowsum = smp.tile([P, 1], f32, tag="rs", bufs=2)
            nc.vector.reduce_sum(rowsum, e_sb, axis=mybir.AxisListType.X)
            z_ps = psum.tile([1, 1], f32, tag="sm", bufs=1)
            nc.tensor.matmul(z_ps, rowsum, ones_p1, start=True, stop=True)
            zsb = smp.tile([1, 1], f32, tag="zsb", bufs=2)
            nc.vector.tensor_copy(zsb, z_ps)
            rz = smp.tile([1, 1], f32, tag="rz", bufs=2)
            nc.vector.reciprocal(rz, zsb)
            rzb_ps = psum.tile([P, 1], f32, tag="sm", bufs=1)
            nc.tensor.matmul(rzb_ps, ones_1p, rz, start=True, stop=True)
            rzb = smp.tile([P, 1], f32, tag="rzb", bufs=2)
            nc.vector.tensor_copy(rzb, rzb_ps)
            aw_sb = smp.tile([P, ST], f32, tag="awsb", bufs=2)
            nc.vector.tensor_scalar_mul(aw_sb, e_sb, rzb)
            nc.sync.dma_start(
                out=aw_t[b].rearrange("(t p) o -> p (t o)", p=P), in_=aw_sb)
            ctx_sb = smp.tile([1, D], f32, tag="ctxsb", bufs=1)
            nc.scalar.activation(ctx_sb, bst["ctx_ps"][0:1, :], AF.Copy,
                                 scale=rz)
            nc.sync.dma_start(out=ctx_t[b:b + 1], in_=ctx_sb)

        batches = [bi for _ in range(repeat) for bi in range(NB)]
        chunk_list = [(b, c) for b in batches for c in range(NCH)]
        prefetched = {0: prefetch_nat0, 1: prefetch_nat1}
        vts_prev = None
        prev_bc = None
        first = True
        bstates = {}
        pend = []   # (b, c) chunks whose score is emitted, ctx pending

        def flush_pend(upto):
            # emit pending ctx chunks; keep at most `upto` outstanding
            while len(pend) > upto:
                pb2, pc2 = pend.pop(0)
                emit_ctx_chunk(pb2, pc2, bstates[pb2])
                if pc2 == NCH - 1:
                    emit_batch_finish(pb2, bstates.pop(pb2))

        for idx, (b, c) in enumerate(chunk_list):
            nat = prefetched.pop(idx, None)
            if nat is None:
                nat = emit_nat_dma(b, c)
            vts_cur = []
            if vts_prev is None:
                for k in range(KD):
                    vts_cur.append(emit_transpose_k(nat, k))
            else:
                pb, pc = prev_bc
                bst = bstates.setdefault(pb, {})
                sc_ps = psum.tile([2, SC], f32, tag="ps", bufs=1)
                for j in range(JU):
                    vts_cur.append(emit_transpose_k(nat, j))
                    emit_mm_j(pb, j, vts_prev, sc_ps)
                emit_score_finish(pb, pc, sc_ps, bst)
                pend.append((pb, pc))
                flush_pend(ctx_lag)
            if first and qproj_late and c == 0:
                emit_qproj()
                tmpp_scope.close()
                first = False
            vts_prev = vts_cur
            prev_bc = (b, c)

        # drain: matmuls of the final chunk, then remaining softmax/ctx.
        pb, pc = prev_bc
        bst = bstates.setdefault(pb, {})
        sc_ps = psum.tile([2, SC], f32, tag="ps", bufs=1)
        for j in range(JU):
            emit_mm_j(pb, j, vts_prev, sc_ps)
        emit_score_finish(pb, pc, sc_ps, bst)
        pend.append((pb, pc))
        flush_pend(0)

    nc.compile()
    _cache[key] = nc
    return nc


def _in_maps(query, values, mask, w1, b1, w2, b2, v, bv):
    query = np.ascontiguousarray(np.asarray(query, dtype=np.float32))
    values = np.ascontiguousarray(np.asarray(values, dtype=np.float32))
    mask = np.ascontiguousarray(np.asarray(mask, dtype=np.int32))
    w1 = np.ascontiguousarray(np.asarray(w1, dtype=np.float32))
    b1 = np.ascontiguousarray(np.asarray(b1, dtype=np.float32))
    w2 = np.ascontiguousarray(np.asarray(w2, dtype=np.float32))
    b2 = np.ascontiguousarray(np.asarray(b2, dtype=np.float32))
    v = np.ascontiguousarray(np.asarray(v, dtype=np.float32))
    bv = np.ascontiguousarray(np.asarray(bv, dtype=np.float32))
    maps = []
    for c in range(NCORES):
        sl = slice(c * NB, (c + 1) * NB)
        maps.append({
            "query": query[sl], "values": values[sl], "mask": mask[sl],
            "w1": w1, "b1": b1, "w2": w2, "b2": b2, "v": v, "bv": bv,
        })
    return maps


def kernel(query, values, mask, w1, b1, w2, b2, v, bv, _trace=False, _mode="f32r"):
    from concourse import bass_utils

    nc = build_module(_mode)
    maps = _in_maps(query, values, mask, w1, b1, w2, b2, v, bv)
    res = bass_utils.run_bass_kernel_spmd(
        nc, maps, core_ids=list(range(NCORES)), trace=_trace)
    ctx = np.concatenate([r["context_out"] for r in res.results], axis=0)
    aw = np.concatenate([r["aw_out"] for r in res.results], axis=0)
    if _trace:
        kernel.last_results = res
    return ctx, aw
